# revision 14
# baseline (speedup 1.0000x reference)
"""Causal self-attention (B=4, T=2048, HID=768, H=12) on 8 NeuronCores.

Sharding: core c handles batch b=c//2 and head-half c%2 (6 of 12 heads).
Data-parallel on B, tensor-parallel on heads; no cross-device communication.

Per-core kernel (all matmuls fp32r = full-rate fp32):
  - host feeds xT=[768,2048] (hidden[b].T) and W.T column slices so every
    matmul has its contraction dim on SBUF partitions.
  - qT/kT = W.T.T @ xT + b, laid out [128=2 heads x 64d, 2048 tok] per pair,
    so the two heads of a pair run score matmuls concurrently in the PE
    array's two 64-row groups (K=64 row tiling).
  - scores are computed transposed, S^T[k, q], per 128-key chunk into a
    [128, 1024] PSUM tile (both heads side by side); one ACT exp per chunk
    covers both heads via a 3D AP, with scale=1/8 and the additive
    attention mask as the per-partition bias (k is the partition dim).
    No max subtraction -- logits are O(1) by construction.
  - causal masking = column-range restriction (only q >= chunk start is
    ever computed/consumed) + triangular zeroing of the diagonal 128x128
    block via gpsimd affine_select on the exp'd tile.
  - V is augmented with a 65th all-ones column so the ctx matmul
    accumulates ctx_num^T = P V and the softmax denominator Z in one
    [65, 512] PSUM tile; normalization = reciprocal_approx_fast(Z) ->
    gpsimd partition_broadcast -> DVE multiply.
  - two (head-pair, q-chunk) units are interleaved chunk-by-chunk so PE
    never waits on ACT exp (keeps the PE HAM clock-gate warm).
  - output is written transposed [384, 2048]; host transposes back.
"""

import sys
from collections import deque

for _p in ("/root/.axon_site/_ro/trn_rl_repo", "/opt/trn_rl_repo"):
    if _p not in sys.path:
        sys.path.append(_p)

import numpy as np

import concourse.bass as bass
import concourse.mybir as mybir
import concourse.tile as tile
from concourse import bacc
from concourse.bass_utils import run_bass_kernel_spmd

F32 = mybir.dt.float32
F32R = mybir.dt.float32r

B, T, HID, H = 4, 2048, 768, 12
D = HID // H            # 64
NH = 6                  # heads per core
NPAIR = 3               # head pairs per core
OC = NH * D             # 384 output dims per core
NCI = HID // 128        # 6 contraction chunks
NJ = T // 512           # 4 query chunks of 512
NT16 = T // 128         # 16 token chunks of 128

_TRACE = False
_TMPDIR = None
LAST_EXEC_NS = None
_COMPILED = None


def _install_trace_hook():
    import types

    if "antenv.axon_hooks" in sys.modules:
        return
    mod = types.ModuleType("antenv.axon_hooks")
    mod._hook = None
    mod.set_axon_ntff_profile_hook = lambda h: setattr(mod, "_hook", h)
    mod.get_axon_ntff_profile_hook = lambda: mod._hook
    sys.modules["antenv.axon_hooks"] = mod
    sys.path.insert(0, "/root/.axon_site")
    from trn_agent_boot.trn_boot import _ntff_profile_via_ctypes

    mod.set_axon_ntff_profile_hook(
        _ntff_profile_via_ctypes("/opt/axon/libaxon_pjrt.so")
    )


class _Unit:
    """One (head-pair, q-chunk-of-512) attention work unit."""

    def __init__(self, pi, j, slot):
        self.pi = pi
        self.j = j
        self.slot = slot
        self.nk = 4 * (j + 1)
        self.kc = 0
        self.pend = deque()
        self.ctx = None


def _build():
    nc = bacc.Bacc("TRN2", target_bir_lowering=False)

    xT = nc.dram_tensor("xT", [HID, T], F32R, kind="ExternalInput")
    wqT = nc.dram_tensor("wqT", [HID, OC], F32R, kind="ExternalInput")
    wkT = nc.dram_tensor("wkT", [HID, OC], F32R, kind="ExternalInput")
    wvT = nc.dram_tensor("wvT", [HID, OC], F32R, kind="ExternalInput")
    bqT = nc.dram_tensor("bqT", [128, NPAIR], F32, kind="ExternalInput")
    bkT = nc.dram_tensor("bkT", [128, NPAIR], F32, kind="ExternalInput")
    bv = nc.dram_tensor("bv", [OC], F32, kind="ExternalInput")
    maskT = nc.dram_tensor("maskT", [128, NT16], F32, kind="ExternalInput")
    outT = nc.dram_tensor("outT", [OC, T], F32, kind="ExternalOutput")

    with tile.TileContext(nc) as tc:
        consts = tc.alloc_tile_pool(name="consts", bufs=1)
        qk_pool = tc.alloc_tile_pool(name="qk", bufs=1)
        va_pool = tc.alloc_tile_pool(name="va", bufs=1)

        # ---- constants ----
        bq_t = consts.tile([128, NPAIR], F32, tag="bq")
        bk_t = consts.tile([128, NPAIR], F32, tag="bk")
        bvr = consts.tile([128, NH, D], F32, tag="bvr")
        mk_t = consts.tile([128, NT16], F32, tag="mk")
        nc.sync.dma_start(out=bq_t, in_=bqT[:, :])
        nc.sync.dma_start(out=bk_t, in_=bkT[:, :])
        nc.gpsimd.dma_start(
            out=bvr,
            in_=bv[:].partition_broadcast(128).rearrange(
                "p (h d) -> p h d", h=NH
            ),
        )
        nc.sync.dma_start(out=mk_t, in_=maskT[:, :])

        # persistent activations
        qT = qk_pool.tile([128, NPAIR, T], F32R, tag="qT")
        kT = qk_pool.tile([128, NPAIR, T], F32R, tag="kT")
        va = va_pool.tile([128, NT16, NH, D + 1], F32R, tag="va")
        ones = consts.tile([128, 1], F32, tag="ones", name="ones")
        nc.vector.memset(ones, 1.0)

        pin_p = tc.alloc_tile_pool(name="pin", bufs=1)
        xt = pin_p.tile([128, NCI, T], F32R, tag="xt")
        wq_t = pin_p.tile([128, NCI, OC], F32R, tag="wq")
        wk_t = pin_p.tile([128, NCI, OC], F32R, tag="wk")
        wv_t = pin_p.tile([128, NCI, OC], F32R, tag="wv")
        for ci in range(NCI):
            nc.sync.dma_start(out=xt[:, ci, :], in_=xT[128 * ci:128 * (ci + 1), :])
            nc.sync.dma_start(out=wq_t[:, ci, :], in_=wqT[128 * ci:128 * (ci + 1), :])
        for ci in range(NCI):
            nc.sync.dma_start(out=wk_t[:, ci, :], in_=wkT[128 * ci:128 * (ci + 1), :])
        for ci in range(NCI):
            nc.sync.dma_start(out=wv_t[:, ci, :], in_=wvT[128 * ci:128 * (ci + 1), :])

        # warm-up operands for HAM filler matmuls (no DMA dependency)
        warm_f = consts.tile([128, 512], F32, tag="warmf", name="warmf")
        nc.vector.memset(warm_f, 0.0)
        warm = consts.tile([128, 512], F32R, tag="warm", name="warm")
        nc.vector.tensor_copy(warm, warm_f)

        pps = tc.alloc_tile_pool(name="pps", bufs=2, space="PSUM")
        sp = tc.alloc_tile_pool(name="sp", bufs=2, space="PSUM")
        cx = tc.alloc_tile_pool(name="cx", bufs=1, space="PSUM")
        pt_pool = tc.alloc_tile_pool(name="pt", bufs=4)
        npool = tc.alloc_tile_pool(name="np", bufs=2)

        # ---- projection work units (emitted lazily, interleaved with
        # attention so the PE stays dense while ACT chews on exps) ----
        def qk_chain(w_t, b_t, dst, pi, tj):
            def emit():
                ps = pps.tile([128, 512], F32, tag="ps", name="ps")
                for ci in range(NCI):
                    nc.tensor.matmul(
                        ps,
                        w_t[:, ci, 128 * pi:128 * (pi + 1)],
                        xt[:, ci, 512 * tj:512 * (tj + 1)],
                        start=(ci == 0),
                        stop=(ci == NCI - 1),
                    )
                nc.vector.tensor_scalar_add(
                    dst[:, pi, 512 * tj:512 * (tj + 1)], ps, b_t[:, pi:pi + 1]
                )
            return emit

        def v_chain(t16):
            def emit():
                ps = pps.tile([128, OC], F32, tag="ps", name="ps")
                for ci in range(NCI):
                    nc.tensor.matmul(
                        ps,
                        xt[:, ci, 128 * t16:128 * (t16 + 1)],
                        wv_t[:, ci, :],
                        start=(ci == 0),
                        stop=(ci == NCI - 1),
                    )
                nc.vector.tensor_tensor(
                    va[:, t16, :, 0:D],
                    ps.rearrange("p (h d) -> p h d", h=NH),
                    bvr,
                    op=mybir.AluOpType.add,
                )
                nc.vector.tensor_copy(va[:, t16, :, D], ones.to_broadcast([128, NH]))
            return emit

        chains = {}
        order = []
        for pi in range(NPAIR):
            for tj in range(NJ):
                chains[f"q{pi}{tj}"] = qk_chain(wq_t, bq_t, qT, pi, tj)
                chains[f"k{pi}{tj}"] = qk_chain(wk_t, bk_t, kT, pi, tj)
        for t16 in range(NT16):
            chains[f"v{t16}"] = v_chain(t16)
        for pi in range(NPAIR):
            for tj in range(NJ):
                order.append(f"q{pi}{tj}")
                order.append(f"k{pi}{tj}")
                if pi == 0:
                    for t16 in range(4 * tj, 4 * tj + 4):
                        order.append(f"v{t16}")
        pending = deque(order)
        done = set()

        def emit_chain(name):
            if name not in done:
                done.add(name)
                chains[name]()

        def filler():
            wp = sp.tile([128, 2, 512], F32, tag="s", name="s2")
            nc.tensor.matmul(wp[:, 0, :], warm[:, 0:128], warm,
                             start=True, stop=True)

        def pop_chain():
            while pending and pending[0] in done:
                pending.popleft()
            if pending:
                emit_chain(pending.popleft())
            else:
                filler()

        # HAM warm-up: keep the PE busy while input DMAs stream in
        for _ in range(24):
            wp = sp.tile([128, 2, 512], F32, tag="s", name="s2")
            nc.tensor.matmul(wp[:, 0, :], warm[:, 0:128], warm,
                             start=True, stop=True)

        # ---- attention ----
        def emit_step(u, step_i):
            kc = u.kc
            u.kc += 1
            c0 = max(0, kc - 4 * u.j) * 128
            s2 = sp.tile([128, 2, 512], F32, tag="s", name="s2")
            for half in range(2):
                rows = slice(64 * half, 64 * half + 64)
                nc.tensor.matmul(
                    s2[:, half, c0:],
                    kT[rows, u.pi, 128 * kc:128 * (kc + 1)],
                    qT[rows, u.pi, 512 * u.j + c0:512 * (u.j + 1)],
                    start=True, stop=True,
                )
            pt = pt_pool.tile([128, 2, 512], F32R, tag="pt", name="pt")
            nc.scalar.activation(
                pt[:, :, c0:], s2[:, :, c0:],
                mybir.ActivationFunctionType.Exp,
                bias=mk_t[:, kc:kc + 1], scale=0.125,
            )
            if kc >= 4 * u.j:  # diagonal chunk: zero below-diagonal
                for half in range(2):
                    nc.gpsimd.affine_select(
                        out=pt[:, half, c0:c0 + 128],
                        in_=pt[:, half, c0:c0 + 128],
                        compare_op=mybir.AluOpType.is_ge,
                        fill=0.0,
                        base=0,
                        pattern=[[1, 128]],
                        channel_multiplier=-1,
                    )
            if step_i % 3 == 2:
                pop_chain()
            u.pend.append((kc, c0, pt))
            if len(u.pend) > 1:
                emit_ctx(u, u.pend.popleft())

        def emit_ctx(u, item):
            kc, c0, pt = item
            for half in range(2):
                nc.tensor.matmul(
                    u.ctx[half][:, c0:],
                    va[:, kc, 2 * u.pi + half, :],
                    pt[:, half, c0:],
                    start=(kc == 0),
                    stop=(kc == u.nk - 1),
                )

        def emit_norm(u):
            for half in range(2):
                hl = 2 * u.pi + half
                zr = npool.tile([1, 512], F32, tag="zr", name="zr")
                nc.vector.tensor_copy(zr, u.ctx[half][D:D + 1, :])
                zrec = npool.tile([1, 512], F32, tag="zrec", name="zrec")
                nc.vector.reciprocal_approx_fast(zrec, zr)
                zrep = npool.tile([64, 512], F32, tag="zrep", name="zrep")
                nc.gpsimd.partition_broadcast(zrep, zrec)
                ot = npool.tile([64, 512], F32, tag="ot", name="ot")
                nc.vector.tensor_tensor(
                    ot, u.ctx[half][0:D, :], zrep, op=mybir.AluOpType.mult
                )
                nc.sync.dma_start(
                    out=outT[D * hl:D * (hl + 1), 512 * u.j:512 * (u.j + 1)],
                    in_=ot,
                )

        step_i = 0
        for pi in range(NPAIR):
            for j in range(NJ):
                u = _Unit(pi, j, 0)
                # prerequisites for this unit
                emit_chain(f"q{pi}{j}")
                for tj in range(j + 1):
                    emit_chain(f"k{pi}{tj}")
                for t16 in range(4 * (j + 1)):
                    emit_chain(f"v{t16}")
                u.ctx = (
                    cx.tile([D + 1, 512], F32, tag="ca", name="ctxa"),
                    cx.tile([D + 1, 512], F32, tag="cb", name="ctxb"),
                )
                while u.kc < u.nk:
                    emit_step(u, step_i)
                    step_i += 1
                while u.pend:
                    emit_ctx(u, u.pend.popleft())
                emit_norm(u)
        while pending:
            pop_chain()

        npool.release()
        pt_pool.release()
        cx.release()
        sp.release()
        pps.release()
        pin_p.release()
        va_pool.release()
        qk_pool.release()
        consts.release()

    nc.compile()
    return nc


def kernel(**inputs):
    global _COMPILED, LAST_EXEC_NS
    hs = np.asarray(inputs["hidden_states"], dtype=np.float32)
    am = np.asarray(inputs["attention_mask"], dtype=np.float32)
    Wq = np.asarray(inputs["Wq"], dtype=np.float32)
    bq = np.asarray(inputs["bq"], dtype=np.float32)
    Wk = np.asarray(inputs["Wk"], dtype=np.float32)
    bk = np.asarray(inputs["bk"], dtype=np.float32)
    Wv = np.asarray(inputs["Wv"], dtype=np.float32)
    bv = np.asarray(inputs["bv"], dtype=np.float32)

    if _COMPILED is None:
        _COMPILED = _build()
    nc = _COMPILED

    c = np.ascontiguousarray
    in_maps = []
    for core in range(8):
        b, half = core // 2, core % 2
        o0 = OC * half
        sl = slice(o0, o0 + OC)
        in_maps.append({
            "xT": c(hs[b].T),                                  # [768, 2048]
            "wqT": c(Wq[sl, :].T),                             # [768, 384]
            "wkT": c(Wk[sl, :].T),
            "wvT": c(Wv[sl, :].T),
            "bqT": c(bq[sl].reshape(NPAIR, 128).T),
            "bkT": c(bk[sl].reshape(NPAIR, 128).T),
            "bv": c(bv[sl]),
            "maskT": c(am[b, 0, 0, :].reshape(NT16, 128).T),
        })

    if _TRACE:
        _install_trace_hook()
    res = run_bass_kernel_spmd(
        nc, in_maps, list(range(8)), trace=_TRACE, tmpdir=_TMPDIR
    )
    LAST_EXEC_NS = res.exec_time_ns

    out = np.empty((B, T, HID), dtype=np.float32)
    for core in range(8):
        b, half = core // 2, core % 2
        out[b, :, OC * half:OC * (half + 1)] = res.results[core]["outT"].T
    return out


# revision 15
# speedup vs baseline: 1.0927x; 1.0927x over previous
"""Causal self-attention (B=4, T=2048, HID=768, H=12) on 8 NeuronCores.

Sharding: core c handles batch b=c//2 and head-half c%2 (6 of 12 heads).
Data-parallel on B, tensor-parallel on heads; no cross-device communication.

Per-core kernel (all matmuls fp32r = full-rate fp32):
  - host feeds xT=[768,2048] (hidden[b].T) and W.T column slices so every
    matmul has its contraction dim on SBUF partitions.
  - qT/kT = W.T.T @ xT + b, laid out [128=2 heads x 64d, 2048 tok] per pair,
    so the two heads of a pair run score matmuls concurrently in the PE
    array's two 64-row groups (K=64 row tiling).
  - scores are computed transposed, S^T[k, q], per 128-key chunk into a
    [128, 1024] PSUM tile (both heads side by side); one ACT exp per chunk
    covers both heads via a 3D AP, with scale=1/8 and the additive
    attention mask as the per-partition bias (k is the partition dim).
    No max subtraction -- logits are O(1) by construction.
  - causal masking = column-range restriction (only q >= chunk start is
    ever computed/consumed) + triangular zeroing of the diagonal 128x128
    block via gpsimd affine_select on the exp'd tile.
  - V is augmented with a 65th all-ones column so the ctx matmul
    accumulates ctx_num^T = P V and the softmax denominator Z in one
    [65, 512] PSUM tile; normalization = reciprocal_approx_fast(Z) ->
    gpsimd partition_broadcast -> DVE multiply.
  - two (head-pair, q-chunk) units are interleaved chunk-by-chunk so PE
    never waits on ACT exp (keeps the PE HAM clock-gate warm).
  - output is written transposed [384, 2048]; host transposes back.
"""

import sys
from collections import deque

for _p in ("/root/.axon_site/_ro/trn_rl_repo", "/opt/trn_rl_repo"):
    if _p not in sys.path:
        sys.path.append(_p)

import numpy as np

import concourse.bass as bass
import concourse.mybir as mybir
import concourse.tile as tile
from concourse import bacc
from concourse.bass_utils import run_bass_kernel_spmd

F32 = mybir.dt.float32
F32R = mybir.dt.float32r

B, T, HID, H = 4, 2048, 768, 12
D = HID // H            # 64
NH = 6                  # heads per core
NPAIR = 3               # head pairs per core
OC = NH * D             # 384 output dims per core
NCI = HID // 128        # 6 contraction chunks
NJ = T // 512           # 4 query chunks of 512
NT16 = T // 128         # 16 token chunks of 128

_TRACE = False
_TMPDIR = None
LAST_EXEC_NS = None
_COMPILED = None


def _install_trace_hook():
    import types

    if "antenv.axon_hooks" in sys.modules:
        return
    mod = types.ModuleType("antenv.axon_hooks")
    mod._hook = None
    mod.set_axon_ntff_profile_hook = lambda h: setattr(mod, "_hook", h)
    mod.get_axon_ntff_profile_hook = lambda: mod._hook
    sys.modules["antenv.axon_hooks"] = mod
    sys.path.insert(0, "/root/.axon_site")
    from trn_agent_boot.trn_boot import _ntff_profile_via_ctypes

    mod.set_axon_ntff_profile_hook(
        _ntff_profile_via_ctypes("/opt/axon/libaxon_pjrt.so")
    )


class _Unit:
    """One (head-pair, q-chunk-of-512) attention work unit."""

    def __init__(self, pi, j, slot):
        self.pi = pi
        self.j = j
        self.slot = slot
        self.nk = 4 * (j + 1)
        self.kc = 0
        self.pend = deque()
        self.ctx = None


def _build():
    nc = bacc.Bacc("TRN2", target_bir_lowering=False)

    xT = nc.dram_tensor("xT", [HID, T], F32R, kind="ExternalInput")
    wqT = nc.dram_tensor("wqT", [HID, OC], F32R, kind="ExternalInput")
    wkT = nc.dram_tensor("wkT", [HID, OC], F32R, kind="ExternalInput")
    wvT = nc.dram_tensor("wvT", [HID, OC], F32R, kind="ExternalInput")
    bqT = nc.dram_tensor("bqT", [128, NPAIR], F32, kind="ExternalInput")
    bkT = nc.dram_tensor("bkT", [128, NPAIR], F32, kind="ExternalInput")
    bv = nc.dram_tensor("bv", [OC], F32, kind="ExternalInput")
    maskT = nc.dram_tensor("maskT", [128, NT16], F32, kind="ExternalInput")
    outT = nc.dram_tensor("outT", [OC, T], F32, kind="ExternalOutput")

    with tile.TileContext(nc) as tc:
        consts = tc.alloc_tile_pool(name="consts", bufs=1)
        qk_pool = tc.alloc_tile_pool(name="qk", bufs=1)
        va_pool = tc.alloc_tile_pool(name="va", bufs=1)

        # ---- constants ----
        bq_t = consts.tile([128, NPAIR], F32, tag="bq")
        bk_t = consts.tile([128, NPAIR], F32, tag="bk")
        bvr = consts.tile([128, NH, D], F32, tag="bvr")
        mk_t = consts.tile([128, NT16], F32, tag="mk")
        nc.sync.dma_start(out=bq_t, in_=bqT[:, :])
        nc.sync.dma_start(out=bk_t, in_=bkT[:, :])
        nc.gpsimd.dma_start(
            out=bvr,
            in_=bv[:].partition_broadcast(128).rearrange(
                "p (h d) -> p h d", h=NH
            ),
        )
        nc.sync.dma_start(out=mk_t, in_=maskT[:, :])

        # persistent activations
        qT = qk_pool.tile([128, NPAIR, T], F32R, tag="qT")
        kT = qk_pool.tile([128, NPAIR, T], F32R, tag="kT")
        va = va_pool.tile([128, NT16, NH, D + 1], F32R, tag="va")
        ones = consts.tile([128, 1], F32, tag="ones", name="ones")
        nc.vector.memset(ones, 1.0)

        pin_p = tc.alloc_tile_pool(name="pin", bufs=1)
        xt = pin_p.tile([128, NCI, T], F32R, tag="xt")
        wq_t = pin_p.tile([128, NCI, OC], F32R, tag="wq")
        wk_t = pin_p.tile([128, NCI, OC], F32R, tag="wk")
        wv_t = pin_p.tile([128, NCI, OC], F32R, tag="wv")
        for ci in range(NCI):
            nc.sync.dma_start(
                out=xt[:, ci, 0:512], in_=xT[128 * ci:128 * (ci + 1), 0:512]
            )
            nc.sync.dma_start(out=wq_t[:, ci, :], in_=wqT[128 * ci:128 * (ci + 1), :])
        for ci in range(NCI):
            nc.sync.dma_start(out=wk_t[:, ci, :], in_=wkT[128 * ci:128 * (ci + 1), :])
        for ci in range(NCI):
            nc.sync.dma_start(out=wv_t[:, ci, :], in_=wvT[128 * ci:128 * (ci + 1), :])
        for tj in range(1, NJ):
            for ci in range(NCI):
                nc.sync.dma_start(
                    out=xt[:, ci, 512 * tj:512 * (tj + 1)],
                    in_=xT[128 * ci:128 * (ci + 1), 512 * tj:512 * (tj + 1)],
                )

        # warm-up operands for HAM filler matmuls (no DMA dependency)
        warm_f = consts.tile([128, 512], F32, tag="warmf", name="warmf")
        nc.vector.memset(warm_f, 0.0)
        warm = consts.tile([128, 512], F32R, tag="warm", name="warm")
        nc.vector.tensor_copy(warm, warm_f)

        pps = tc.alloc_tile_pool(name="pps", bufs=2, space="PSUM")
        sp = tc.alloc_tile_pool(name="sp", bufs=2, space="PSUM")
        cx = tc.alloc_tile_pool(name="cx", bufs=1, space="PSUM")
        pt_pool = tc.alloc_tile_pool(name="pt", bufs=6)
        npool = tc.alloc_tile_pool(name="np", bufs=2)

        # ---- projection work units (emitted lazily, interleaved with
        # attention so the PE stays dense while ACT chews on exps) ----
        def qk_chain(w_t, b_t, dst, pi, tj):
            def emit():
                ps = pps.tile([128, 512], F32, tag="ps", name="ps")
                for ci in range(NCI):
                    nc.tensor.matmul(
                        ps,
                        w_t[:, ci, 128 * pi:128 * (pi + 1)],
                        xt[:, ci, 512 * tj:512 * (tj + 1)],
                        start=(ci == 0),
                        stop=(ci == NCI - 1),
                    )
                nc.vector.tensor_scalar_add(
                    dst[:, pi, 512 * tj:512 * (tj + 1)], ps, b_t[:, pi:pi + 1]
                )
            return emit

        def v_chain(t16):
            def emit():
                ps = pps.tile([128, OC], F32, tag="ps", name="ps")
                for ci in range(NCI):
                    nc.tensor.matmul(
                        ps,
                        xt[:, ci, 128 * t16:128 * (t16 + 1)],
                        wv_t[:, ci, :],
                        start=(ci == 0),
                        stop=(ci == NCI - 1),
                    )
                nc.vector.tensor_tensor(
                    va[:, t16, :, 0:D],
                    ps.rearrange("p (h d) -> p h d", h=NH),
                    bvr,
                    op=mybir.AluOpType.add,
                )
                nc.vector.tensor_copy(va[:, t16, :, D], ones.to_broadcast([128, NH]))
            return emit

        chains = {}
        order = []
        for pi in range(NPAIR):
            for tj in range(NJ):
                chains[f"q{pi}{tj}"] = qk_chain(wq_t, bq_t, qT, pi, tj)
                chains[f"k{pi}{tj}"] = qk_chain(wk_t, bk_t, kT, pi, tj)
        for t16 in range(NT16):
            chains[f"v{t16}"] = v_chain(t16)
        for tj in range(NJ):
            for pi in range(NPAIR):
                order.append(f"q{pi}{tj}")
                order.append(f"k{pi}{tj}")
                if pi == 0:
                    for t16 in range(4 * tj, 4 * tj + 4):
                        order.append(f"v{t16}")
        pending = deque(order)
        done = set()

        def emit_chain(name):
            if name not in done:
                done.add(name)
                chains[name]()

        def filler():
            wp = sp.tile([128, 2, 512], F32, tag="s", name="s2")
            nc.tensor.matmul(wp[:, 0, :], warm[:, 0:128], warm,
                             start=True, stop=True)

        def pop_chain():
            while pending and pending[0] in done:
                pending.popleft()
            if pending:
                emit_chain(pending.popleft())
            else:
                filler()

        # HAM warm-up: keep the PE busy while input DMAs stream in
        for _ in range(24):
            wp = sp.tile([128, 2, 512], F32, tag="s", name="s2")
            nc.tensor.matmul(wp[:, 0, :], warm[:, 0:128], warm,
                             start=True, stop=True)

        # ---- attention ----
        def emit_step(u, step_i):
            kc = u.kc
            u.kc += 1
            c0 = max(0, kc - 4 * u.j) * 128
            s2 = sp.tile([128, 2, 512], F32, tag="s", name="s2")
            for half in range(2):
                rows = slice(64 * half, 64 * half + 64)
                nc.tensor.matmul(
                    s2[:, half, c0:],
                    kT[rows, u.pi, 128 * kc:128 * (kc + 1)],
                    qT[rows, u.pi, 512 * u.j + c0:512 * (u.j + 1)],
                    start=True, stop=True,
                )
            pt = pt_pool.tile([128, 2, 512], F32R, tag="pt", name="pt")
            nc.scalar.activation(
                pt[:, :, c0:], s2[:, :, c0:],
                mybir.ActivationFunctionType.Exp,
                bias=mk_t[:, kc:kc + 1], scale=0.125,
            )
            if kc >= 4 * u.j:  # diagonal chunk: zero below-diagonal
                for half in range(2):
                    nc.gpsimd.affine_select(
                        out=pt[:, half, c0:c0 + 128],
                        in_=pt[:, half, c0:c0 + 128],
                        compare_op=mybir.AluOpType.is_ge,
                        fill=0.0,
                        base=0,
                        pattern=[[1, 128]],
                        channel_multiplier=-1,
                    )
            if step_i % 3 == 2:
                pop_chain()
            u.pend.append((kc, c0, pt))
            if len(u.pend) > 1:
                emit_ctx(u, u.pend.popleft())

        def emit_ctx(u, item):
            kc, c0, pt = item
            for half in range(2):
                nc.tensor.matmul(
                    u.ctx[half][:, c0:],
                    va[:, kc, 2 * u.pi + half, :],
                    pt[:, half, c0:],
                    start=(kc == 0),
                    stop=(kc == u.nk - 1),
                )

        def emit_norm(u):
            for half in range(2):
                hl = 2 * u.pi + half
                zr = npool.tile([1, 512], F32, tag="zr", name="zr")
                nc.vector.tensor_copy(zr, u.ctx[half][D:D + 1, :])
                zrec = npool.tile([1, 512], F32, tag="zrec", name="zrec")
                nc.vector.reciprocal_approx_fast(zrec, zr)
                zrep = npool.tile([64, 512], F32, tag="zrep", name="zrep")
                nc.gpsimd.partition_broadcast(zrep, zrec)
                ot = npool.tile([64, 512], F32, tag="ot", name="ot")
                nc.vector.tensor_tensor(
                    ot, u.ctx[half][0:D, :], zrep, op=mybir.AluOpType.mult
                )
                nc.sync.dma_start(
                    out=outT[D * hl:D * (hl + 1), 512 * u.j:512 * (u.j + 1)],
                    in_=ot,
                )

        step_i = 0
        for j in range(NJ):
            for pi in range(NPAIR):
                u = _Unit(pi, j, 0)
                # prerequisites for this unit
                emit_chain(f"q{pi}{j}")
                for tj in range(j + 1):
                    emit_chain(f"k{pi}{tj}")
                for t16 in range(4 * (j + 1)):
                    emit_chain(f"v{t16}")
                u.ctx = (
                    cx.tile([D + 1, 512], F32, tag="ca", name="ctxa"),
                    cx.tile([D + 1, 512], F32, tag="cb", name="ctxb"),
                )
                while u.kc < u.nk:
                    emit_step(u, step_i)
                    step_i += 1
                while u.pend:
                    emit_ctx(u, u.pend.popleft())
                emit_norm(u)
        while pending:
            pop_chain()

        npool.release()
        pt_pool.release()
        cx.release()
        sp.release()
        pps.release()
        pin_p.release()
        va_pool.release()
        qk_pool.release()
        consts.release()

    nc.compile()
    return nc


def kernel(**inputs):
    global _COMPILED, LAST_EXEC_NS
    hs = np.asarray(inputs["hidden_states"], dtype=np.float32)
    am = np.asarray(inputs["attention_mask"], dtype=np.float32)
    Wq = np.asarray(inputs["Wq"], dtype=np.float32)
    bq = np.asarray(inputs["bq"], dtype=np.float32)
    Wk = np.asarray(inputs["Wk"], dtype=np.float32)
    bk = np.asarray(inputs["bk"], dtype=np.float32)
    Wv = np.asarray(inputs["Wv"], dtype=np.float32)
    bv = np.asarray(inputs["bv"], dtype=np.float32)

    if _COMPILED is None:
        _COMPILED = _build()
    nc = _COMPILED

    c = np.ascontiguousarray
    in_maps = []
    for core in range(8):
        b, half = core // 2, core % 2
        o0 = OC * half
        sl = slice(o0, o0 + OC)
        in_maps.append({
            "xT": c(hs[b].T),                                  # [768, 2048]
            "wqT": c(Wq[sl, :].T),                             # [768, 384]
            "wkT": c(Wk[sl, :].T),
            "wvT": c(Wv[sl, :].T),
            "bqT": c(bq[sl].reshape(NPAIR, 128).T),
            "bkT": c(bk[sl].reshape(NPAIR, 128).T),
            "bv": c(bv[sl]),
            "maskT": c(am[b, 0, 0, :].reshape(NT16, 128).T),
        })

    if _TRACE:
        _install_trace_hook()
    res = run_bass_kernel_spmd(
        nc, in_maps, list(range(8)), trace=_TRACE, tmpdir=_TMPDIR
    )
    LAST_EXEC_NS = res.exec_time_ns

    out = np.empty((B, T, HID), dtype=np.float32)
    for core in range(8):
        b, half = core // 2, core % 2
        out[b, :, OC * half:OC * (half + 1)] = res.results[core]["outT"].T
    return out


# revision 16
# speedup vs baseline: 1.1305x; 1.0346x over previous
"""Causal self-attention (B=4, T=2048, HID=768, H=12) on 8 NeuronCores.

Sharding: core c handles batch b=c//2 and head-half c%2 (6 of 12 heads).
Data-parallel on B, tensor-parallel on heads; no cross-device communication.

Per-core kernel (all matmuls fp32r = full-rate fp32):
  - host feeds xT=[768,2048] (hidden[b].T) and W.T column slices so every
    matmul has its contraction dim on SBUF partitions.
  - qT/kT = W.T.T @ xT + b, laid out [128=2 heads x 64d, 2048 tok] per pair,
    so the two heads of a pair run score matmuls concurrently in the PE
    array's two 64-row groups (K=64 row tiling).
  - scores are computed transposed, S^T[k, q], per 128-key chunk into a
    [128, 1024] PSUM tile (both heads side by side); one ACT exp per chunk
    covers both heads via a 3D AP, with scale=1/8 and the additive
    attention mask as the per-partition bias (k is the partition dim).
    No max subtraction -- logits are O(1) by construction.
  - causal masking = column-range restriction (only q >= chunk start is
    ever computed/consumed) + triangular zeroing of the diagonal 128x128
    block via gpsimd affine_select on the exp'd tile.
  - V is augmented with a 65th all-ones column so the ctx matmul
    accumulates ctx_num^T = P V and the softmax denominator Z in one
    [65, 512] PSUM tile; normalization = reciprocal_approx_fast(Z) ->
    gpsimd partition_broadcast -> DVE multiply.
  - two (head-pair, q-chunk) units are interleaved chunk-by-chunk so PE
    never waits on ACT exp (keeps the PE HAM clock-gate warm).
  - output is written transposed [384, 2048]; host transposes back.
"""

import sys
from collections import deque

for _p in ("/root/.axon_site/_ro/trn_rl_repo", "/opt/trn_rl_repo"):
    if _p not in sys.path:
        sys.path.append(_p)

import numpy as np

import concourse.bass as bass
import concourse.mybir as mybir
import concourse.tile as tile
from concourse import bacc
from concourse.bass_utils import run_bass_kernel_spmd

F32 = mybir.dt.float32
F32R = mybir.dt.float32r

B, T, HID, H = 4, 2048, 768, 12
D = HID // H            # 64
NH = 6                  # heads per core
NPAIR = 3               # head pairs per core
OC = NH * D             # 384 output dims per core
NCI = HID // 128        # 6 contraction chunks
NJ = T // 512           # 4 query chunks of 512
NT16 = T // 128         # 16 token chunks of 128

_TRACE = False
_TMPDIR = None
LAST_EXEC_NS = None
_COMPILED = None


def _install_trace_hook():
    import types

    if "antenv.axon_hooks" in sys.modules:
        return
    mod = types.ModuleType("antenv.axon_hooks")
    mod._hook = None
    mod.set_axon_ntff_profile_hook = lambda h: setattr(mod, "_hook", h)
    mod.get_axon_ntff_profile_hook = lambda: mod._hook
    sys.modules["antenv.axon_hooks"] = mod
    sys.path.insert(0, "/root/.axon_site")
    from trn_agent_boot.trn_boot import _ntff_profile_via_ctypes

    mod.set_axon_ntff_profile_hook(
        _ntff_profile_via_ctypes("/opt/axon/libaxon_pjrt.so")
    )


class _Unit:
    """One (head-pair, q-chunk-of-512) attention work unit."""

    def __init__(self, pi, j, slot):
        self.pi = pi
        self.j = j
        self.slot = slot
        self.nk = 4 * (j + 1)
        self.kc = 0
        self.sq = deque()    # scores awaiting exp (1-step delay)
        self.pend = deque()  # exp'd tiles awaiting ctx (1-step delay)
        self.ctx = None


def _build():
    nc = bacc.Bacc("TRN2", target_bir_lowering=False)

    xT = nc.dram_tensor("xT", [HID, T], F32R, kind="ExternalInput")
    wqT = nc.dram_tensor("wqT", [HID, OC], F32R, kind="ExternalInput")
    wkT = nc.dram_tensor("wkT", [HID, OC], F32R, kind="ExternalInput")
    wvT = nc.dram_tensor("wvT", [HID, OC], F32R, kind="ExternalInput")
    bqT = nc.dram_tensor("bqT", [128, NPAIR], F32, kind="ExternalInput")
    bkT = nc.dram_tensor("bkT", [128, NPAIR], F32, kind="ExternalInput")
    bv = nc.dram_tensor("bv", [OC], F32, kind="ExternalInput")
    maskT = nc.dram_tensor("maskT", [128, NT16], F32, kind="ExternalInput")
    outT = nc.dram_tensor("outT", [OC, T], F32, kind="ExternalOutput")

    with tile.TileContext(nc) as tc:
        consts = tc.alloc_tile_pool(name="consts", bufs=1)
        qk_pool = tc.alloc_tile_pool(name="qk", bufs=1)
        va_pool = tc.alloc_tile_pool(name="va", bufs=1)

        # ---- constants ----
        bq_t = consts.tile([128, NPAIR], F32, tag="bq")
        bk_t = consts.tile([128, NPAIR], F32, tag="bk")
        bvr = consts.tile([128, NH, D], F32, tag="bvr")
        mk_t = consts.tile([128, NT16], F32, tag="mk")
        nc.sync.dma_start(out=bq_t, in_=bqT[:, :])
        nc.sync.dma_start(out=bk_t, in_=bkT[:, :])
        nc.gpsimd.dma_start(
            out=bvr,
            in_=bv[:].partition_broadcast(128).rearrange(
                "p (h d) -> p h d", h=NH
            ),
        )
        nc.sync.dma_start(out=mk_t, in_=maskT[:, :])

        # persistent activations
        qT = qk_pool.tile([128, NPAIR, T], F32R, tag="qT")
        kT = qk_pool.tile([128, NPAIR, T], F32R, tag="kT")
        va = va_pool.tile([128, NT16, NH, D + 1], F32R, tag="va")
        ones = consts.tile([128, 1], F32, tag="ones", name="ones")
        nc.vector.memset(ones, 1.0)

        pin_p = tc.alloc_tile_pool(name="pin", bufs=1)
        xt = pin_p.tile([128, NCI, T], F32R, tag="xt")
        wq_t = pin_p.tile([128, NCI, OC], F32R, tag="wq")
        wk_t = pin_p.tile([128, NCI, OC], F32R, tag="wk")
        wv_t = pin_p.tile([128, NCI, OC], F32R, tag="wv")
        for ci in range(NCI):
            nc.sync.dma_start(
                out=xt[:, ci, 0:512], in_=xT[128 * ci:128 * (ci + 1), 0:512]
            )
            nc.sync.dma_start(out=wq_t[:, ci, :], in_=wqT[128 * ci:128 * (ci + 1), :])
        for ci in range(NCI):
            nc.sync.dma_start(out=wk_t[:, ci, :], in_=wkT[128 * ci:128 * (ci + 1), :])
        for ci in range(NCI):
            nc.sync.dma_start(out=wv_t[:, ci, :], in_=wvT[128 * ci:128 * (ci + 1), :])
        for tj in range(1, NJ):
            for ci in range(NCI):
                nc.sync.dma_start(
                    out=xt[:, ci, 512 * tj:512 * (tj + 1)],
                    in_=xT[128 * ci:128 * (ci + 1), 512 * tj:512 * (tj + 1)],
                )

        # warm-up operands for HAM filler matmuls (no DMA dependency)
        warm_f = consts.tile([128, 512], F32, tag="warmf", name="warmf")
        nc.vector.memset(warm_f, 0.0)
        warm = consts.tile([128, 512], F32R, tag="warm", name="warm")
        nc.vector.tensor_copy(warm, warm_f)

        pps = tc.alloc_tile_pool(name="pps", bufs=2, space="PSUM")
        sp = tc.alloc_tile_pool(name="sp", bufs=2, space="PSUM")
        cx = tc.alloc_tile_pool(name="cx", bufs=1, space="PSUM")
        pt_pool = tc.alloc_tile_pool(name="pt", bufs=6)
        npool = tc.alloc_tile_pool(name="np", bufs=2)

        # ---- projection work units (emitted lazily, interleaved with
        # attention so the PE stays dense while ACT chews on exps) ----
        def qk_chain(w_t, b_t, dst, pi, tj):
            def emit():
                ps = pps.tile([128, 512], F32, tag="ps", name="ps")
                for ci in range(NCI):
                    nc.tensor.matmul(
                        ps,
                        w_t[:, ci, 128 * pi:128 * (pi + 1)],
                        xt[:, ci, 512 * tj:512 * (tj + 1)],
                        start=(ci == 0),
                        stop=(ci == NCI - 1),
                    )
                nc.vector.tensor_scalar_add(
                    dst[:, pi, 512 * tj:512 * (tj + 1)], ps, b_t[:, pi:pi + 1]
                )
            return emit

        def v_chain(t16):
            def emit():
                ps = pps.tile([128, OC], F32, tag="ps", name="ps")
                for ci in range(NCI):
                    nc.tensor.matmul(
                        ps,
                        xt[:, ci, 128 * t16:128 * (t16 + 1)],
                        wv_t[:, ci, :],
                        start=(ci == 0),
                        stop=(ci == NCI - 1),
                    )
                nc.vector.tensor_tensor(
                    va[:, t16, :, 0:D],
                    ps.rearrange("p (h d) -> p h d", h=NH),
                    bvr,
                    op=mybir.AluOpType.add,
                )
                nc.vector.tensor_copy(va[:, t16, :, D], ones.to_broadcast([128, NH]))
            return emit

        chains = {}
        order = []
        for pi in range(NPAIR):
            for tj in range(NJ):
                chains[f"q{pi}{tj}"] = qk_chain(wq_t, bq_t, qT, pi, tj)
                chains[f"k{pi}{tj}"] = qk_chain(wk_t, bk_t, kT, pi, tj)
        for t16 in range(NT16):
            chains[f"v{t16}"] = v_chain(t16)
        for tj in range(NJ):
            for pi in range(NPAIR):
                order.append(f"q{pi}{tj}")
                order.append(f"k{pi}{tj}")
                if pi == 0:
                    for t16 in range(4 * tj, 4 * tj + 4):
                        order.append(f"v{t16}")
        pending = deque(order)
        done = set()

        def emit_chain(name):
            if name not in done:
                done.add(name)
                chains[name]()

        def filler():
            wp = sp.tile([128, 2, 512], F32, tag="s", name="s2")
            nc.tensor.matmul(wp[:, 0, :], warm[:, 0:128], warm,
                             start=True, stop=True)

        def pop_chain():
            while pending and pending[0] in done:
                pending.popleft()
            if pending:
                emit_chain(pending.popleft())
            else:
                filler()

        # HAM warm-up: keep the PE busy while input DMAs stream in
        for _ in range(24):
            wp = sp.tile([128, 2, 512], F32, tag="s", name="s2")
            nc.tensor.matmul(wp[:, 0, :], warm[:, 0:128], warm,
                             start=True, stop=True)

        # ---- attention ----
        def emit_scores(u):
            kc = u.kc
            u.kc += 1
            c0 = max(0, kc - 4 * u.j) * 128
            emit_chain(f"k{u.pi}{kc // 4}")
            s2 = sp.tile([128, 2, 512], F32, tag="s", name="s2")
            for half in range(2):
                rows = slice(64 * half, 64 * half + 64)
                nc.tensor.matmul(
                    s2[:, half, c0:],
                    kT[rows, u.pi, 128 * kc:128 * (kc + 1)],
                    qT[rows, u.pi, 512 * u.j + c0:512 * (u.j + 1)],
                    start=True, stop=True,
                )
            u.sq.append((kc, c0, s2))

        def emit_exp(u):
            kc, c0, s2 = u.sq.popleft()
            pt = pt_pool.tile([128, 2, 512], F32R, tag="pt", name="pt")
            nc.scalar.activation(
                pt[:, :, c0:], s2[:, :, c0:],
                mybir.ActivationFunctionType.Exp,
                bias=mk_t[:, kc:kc + 1], scale=0.125,
            )
            if kc >= 4 * u.j:  # diagonal chunk: zero below-diagonal
                for half in range(2):
                    nc.gpsimd.affine_select(
                        out=pt[:, half, c0:c0 + 128],
                        in_=pt[:, half, c0:c0 + 128],
                        compare_op=mybir.AluOpType.is_ge,
                        fill=0.0,
                        base=0,
                        pattern=[[1, 128]],
                        channel_multiplier=-1,
                    )
            u.pend.append((kc, c0, pt))

        def emit_step(u, step_i):
            emit_scores(u)
            if len(u.sq) > 1:
                emit_exp(u)
            if step_i % 3 == 2:
                pop_chain()
            if len(u.pend) > 1:
                emit_ctx(u, u.pend.popleft())

        def emit_ctx(u, item):
            kc, c0, pt = item
            emit_chain(f"v{kc}")
            for half in range(2):
                nc.tensor.matmul(
                    u.ctx[half][:, c0:],
                    va[:, kc, 2 * u.pi + half, :],
                    pt[:, half, c0:],
                    start=(kc == 0),
                    stop=(kc == u.nk - 1),
                )

        def emit_norm(u):
            for half in range(2):
                hl = 2 * u.pi + half
                zr = npool.tile([1, 512], F32, tag="zr", name="zr")
                nc.vector.tensor_copy(zr, u.ctx[half][D:D + 1, :])
                zrec = npool.tile([1, 512], F32, tag="zrec", name="zrec")
                nc.vector.reciprocal_approx_fast(zrec, zr)
                zrep = npool.tile([64, 512], F32, tag="zrep", name="zrep")
                nc.gpsimd.partition_broadcast(zrep, zrec)
                ot = npool.tile([64, 512], F32, tag="ot", name="ot")
                nc.vector.tensor_tensor(
                    ot, u.ctx[half][0:D, :], zrep, op=mybir.AluOpType.mult
                )
                nc.sync.dma_start(
                    out=outT[D * hl:D * (hl + 1), 512 * u.j:512 * (u.j + 1)],
                    in_=ot,
                )

        step_i = 0
        for j in range(NJ):
            for pi in range(NPAIR):
                u = _Unit(pi, j, 0)
                emit_chain(f"q{pi}{j}")
                u.ctx = (
                    cx.tile([D + 1, 512], F32, tag="ca", name="ctxa"),
                    cx.tile([D + 1, 512], F32, tag="cb", name="ctxb"),
                )
                while u.kc < u.nk:
                    emit_step(u, step_i)
                    step_i += 1
                while u.sq:
                    emit_exp(u)
                while u.pend:
                    emit_ctx(u, u.pend.popleft())
                emit_norm(u)
        while pending:
            pop_chain()

        npool.release()
        pt_pool.release()
        cx.release()
        sp.release()
        pps.release()
        pin_p.release()
        va_pool.release()
        qk_pool.release()
        consts.release()

    nc.compile()
    return nc


def kernel(**inputs):
    global _COMPILED, LAST_EXEC_NS
    hs = np.asarray(inputs["hidden_states"], dtype=np.float32)
    am = np.asarray(inputs["attention_mask"], dtype=np.float32)
    Wq = np.asarray(inputs["Wq"], dtype=np.float32)
    bq = np.asarray(inputs["bq"], dtype=np.float32)
    Wk = np.asarray(inputs["Wk"], dtype=np.float32)
    bk = np.asarray(inputs["bk"], dtype=np.float32)
    Wv = np.asarray(inputs["Wv"], dtype=np.float32)
    bv = np.asarray(inputs["bv"], dtype=np.float32)

    if _COMPILED is None:
        _COMPILED = _build()
    nc = _COMPILED

    c = np.ascontiguousarray
    in_maps = []
    for core in range(8):
        b, half = core // 2, core % 2
        o0 = OC * half
        sl = slice(o0, o0 + OC)
        in_maps.append({
            "xT": c(hs[b].T),                                  # [768, 2048]
            "wqT": c(Wq[sl, :].T),                             # [768, 384]
            "wkT": c(Wk[sl, :].T),
            "wvT": c(Wv[sl, :].T),
            "bqT": c(bq[sl].reshape(NPAIR, 128).T),
            "bkT": c(bk[sl].reshape(NPAIR, 128).T),
            "bv": c(bv[sl]),
            "maskT": c(am[b, 0, 0, :].reshape(NT16, 128).T),
        })

    if _TRACE:
        _install_trace_hook()
    res = run_bass_kernel_spmd(
        nc, in_maps, list(range(8)), trace=_TRACE, tmpdir=_TMPDIR
    )
    LAST_EXEC_NS = res.exec_time_ns

    out = np.empty((B, T, HID), dtype=np.float32)
    for core in range(8):
        b, half = core // 2, core % 2
        out[b, :, OC * half:OC * (half + 1)] = res.results[core]["outT"].T
    return out
